# revision 1
# baseline (speedup 1.0000x reference)
"""Single-head attention block (B=8, N=2048, D=768) on 8 Trainium2 NeuronCores.

Strategy: pure data-parallel over the batch dimension — one batch element per
NeuronCore. Each core computes, for its x_b [N, D]:

  q = x@Wq + bq, k = x@Wk + bk, v = x@Wv        (bf16 matmuls, fp32 PSUM accum)
  expT[j, i] = exp((q_i . k_j) / sqrt(D))        (no max-subtraction: scaled
                                                  scores are bounded ~|2.2|)
  outU[i, :] = sum_j expT[j, i] * v_aug[j, :]    (v_aug has a ones column ->
                                                  col D holds the softmax row
                                                  sums)
  out[i, e] = outU[i, e] / outU[i, D] + bv[e]    (v-bias folded to the end:
                                                  softmax rows sum to 1)

Layouts are chosen so the softmax contraction axis (j) always sits on SBUF
partitions and no on-chip transposes are ever needed:
  - qT, kT [D_part, N_free]   (projections computed transposed)
  - scores computed transposed: scoresT[j_part, i_free]
  - v natural [N_part, D_free] which is exactly the AV matmul's moving operand
"""

import math
import sys

import numpy as np

sys.path.insert(0, "/opt/trn_rl_repo")

import ml_dtypes  # noqa: E402

import concourse.bass as bass  # noqa: E402
import concourse.tile as tile  # noqa: E402
from concourse import bacc, mybir  # noqa: E402
from concourse import bass_utils  # noqa: E402

B, N, D = 8, 2048, 768
P = 128
DC = D // P  # 6 chunks of the embedding/contraction dim
NT = N // P  # 16 chunks of the sequence dim
FD = 512  # matmul free-dim tile (one fp32 PSUM bank)
BF16 = mybir.dt.bfloat16
F32 = mybir.dt.float32
INV_SQRT_D = 1.0 / math.sqrt(D)

# Filled by kernel() so a test harness can report the profiled HW time.
LAST_RESULT = None


def _emit(tc, out, xT, wq, wk, wv, bq, bk, bv):
    nc = tc.nc
    Ident = mybir.ActivationFunctionType.Identity
    Copy = mybir.ActivationFunctionType.Copy
    Exp = mybir.ActivationFunctionType.Exp

    with (
        tc.tile_pool(name="const", bufs=1) as const,
        tc.tile_pool(name="data", bufs=1) as data,
        tc.tile_pool(name="expp", bufs=1) as expp,
        tc.tile_pool(name="psum", bufs=2, space="PSUM") as psum,
        tc.tile_pool(name="outp", bufs=3) as outp,
        tc.tile_pool(name="small", bufs=4) as small,
    ):
        # Per-partition bias tiles: bqs[p, o] = bq[o*128 + p]
        bqs = const.tile([P, DC], F32)
        nc.sync.dma_start(bqs[:], bq.rearrange("(o p) -> p o", p=P))
        bks = const.tile([P, DC], F32)
        nc.sync.dma_start(bks[:], bk.rearrange("(o p) -> p o", p=P))
        # bv broadcast across all partitions: bvb[p, e] = bv[e]
        bvb = const.tile([P, D], F32)
        nc.sync.dma_start(
            bvb[:], bass.AP(tensor=bv.tensor, offset=bv.offset, ap=[[0, P], *bv.ap])
        )

        # Persistent activations
        qT = data.tile([P, DC, N], BF16)  # qT[p, o, n] = q[n, o*128+p]
        kT = data.tile([P, DC, N], BF16)
        v = data.tile([P, NT, D + 16], BF16)  # v[p, t, e] = v[t*128+p, e]; col D = 1.0
        expT = expp.tile([P, NT, N], BF16)  # expT[p, t, i] = exp(s[i, t*128+p]/sqrt(D))

        with tc.tile_pool(name="phase1", bufs=1) as ph1:
            xTs = ph1.tile([P, DC, N], BF16)  # xTs[p, o, n] = x[n, o*128+p]
            nc.sync.dma_start(xTs[:], xT.rearrange("(o p) n -> p o n", p=P))
            wqs = ph1.tile([P, DC, D], BF16)  # wqs[p, o, e] = Wq[o*128+p, e]
            nc.sync.dma_start(wqs[:], wq.rearrange("(o p) e -> p o e", p=P))
            wks = ph1.tile([P, DC, D], BF16)
            nc.sync.dma_start(wks[:], wk.rearrange("(o p) e -> p o e", p=P))
            wvs = ph1.tile([P, DC, D], BF16)
            nc.sync.dma_start(wvs[:], wv.rearrange("(o p) e -> p o e", p=P))

            # q, k projections in transposed layout:
            # qT[e, n] = sum_d Wq[d, e] * xT[d, n], then + bq[e] (e on partitions)
            for ws, bs, dst in ((wqs, bqs, qT), (wks, bks, kT)):
                for ec in range(DC):
                    ps = psum.tile([P, N], F32, tag="ps", name="ps")
                    for dc in range(DC):
                        lhsT = ws[:, dc, ec * P : (ec + 1) * P]
                        for nj in range(N // FD):
                            nc.tensor.matmul(
                                ps[:, nj * FD : (nj + 1) * FD],
                                lhsT=lhsT,
                                rhs=xTs[:, dc, nj * FD : (nj + 1) * FD],
                                start=(dc == 0),
                                stop=(dc == DC - 1),
                            )
                    nc.scalar.activation(
                        dst[:, ec, :], ps[:], Ident, bias=bs[:, ec : ec + 1]
                    )

            # v projection in natural layout: v[n, e] = sum_d xT[d, n] * Wv[d, e]
            # (bias deferred to the epilogue). Column D gets 1.0 so the AV
            # matmul also produces softmax row sums.
            for nt in range(NT):
                ps = psum.tile([P, N], F32, tag="ps", name="ps")
                for dc in range(DC):
                    lhsT = xTs[:, dc, nt * P : (nt + 1) * P]
                    nc.tensor.matmul(
                        ps[:, 0:FD],
                        lhsT=lhsT,
                        rhs=wvs[:, dc, 0:FD],
                        start=(dc == 0),
                        stop=(dc == DC - 1),
                    )
                    nc.tensor.matmul(
                        ps[:, FD:D],
                        lhsT=lhsT,
                        rhs=wvs[:, dc, FD:D],
                        start=(dc == 0),
                        stop=(dc == DC - 1),
                    )
                nc.scalar.activation(v[:, nt, 0:D], ps[:, 0:D], Copy)
                nc.vector.memset(v[:, nt, D : D + 1], 1.0)

        # scoresT[j, i] = sum_d kT[d, j] * qT[d, i]; exp with the 1/sqrt(D)
        # scale folded into the activation.
        for jt in range(NT):
            ps = psum.tile([P, N], F32, tag="ps", name="ps")
            for dc in range(DC):
                lhsT = kT[:, dc, jt * P : (jt + 1) * P]
                for ni in range(N // FD):
                    nc.tensor.matmul(
                        ps[:, ni * FD : (ni + 1) * FD],
                        lhsT=lhsT,
                        rhs=qT[:, dc, ni * FD : (ni + 1) * FD],
                        start=(dc == 0),
                        stop=(dc == DC - 1),
                    )
            nc.scalar.activation(expT[:, jt, :], ps[:], Exp, scale=INV_SQRT_D)

        # out[i, e] = sum_j expT[j, i] * v[j, e]; col D accumulates the row sum.
        for it in range(NT):
            ps = psum.tile([P, N], F32, tag="ps", name="ps")
            for jt in range(NT):
                lhsT = expT[:, jt, it * P : (it + 1) * P]
                nc.tensor.matmul(
                    ps[:, 0:FD],
                    lhsT=lhsT,
                    rhs=v[:, jt, 0:FD],
                    start=(jt == 0),
                    stop=(jt == NT - 1),
                )
                nc.tensor.matmul(
                    ps[:, FD : D + 1],
                    lhsT=lhsT,
                    rhs=v[:, jt, FD : D + 1],
                    start=(jt == 0),
                    stop=(jt == NT - 1),
                )
            recip = small.tile([P, 1], F32, tag="recip", name="recip")
            nc.vector.reciprocal(recip[:], ps[:, D : D + 1])
            of = outp.tile([P, D], F32, tag="of", name="of")
            nc.vector.scalar_tensor_tensor(
                of[:],
                ps[:, 0:D],
                recip[:],
                bvb[:],
                op0=mybir.AluOpType.mult,
                op1=mybir.AluOpType.add,
            )
            nc.sync.dma_start(out[it * P : (it + 1) * P, :], of[:])


def _build():
    nc = bacc.Bacc(
        "TRN2",
        target_bir_lowering=False,
        debug=False,
        enable_asserts=False,
        num_devices=B,
    )
    xT = nc.dram_tensor("xT", [D, N], BF16, kind="ExternalInput").ap()
    wq = nc.dram_tensor("wq", [D, D], BF16, kind="ExternalInput").ap()
    wk = nc.dram_tensor("wk", [D, D], BF16, kind="ExternalInput").ap()
    wv = nc.dram_tensor("wv", [D, D], BF16, kind="ExternalInput").ap()
    bq = nc.dram_tensor("bq", [D], F32, kind="ExternalInput").ap()
    bk = nc.dram_tensor("bk", [D], F32, kind="ExternalInput").ap()
    bv = nc.dram_tensor("bv", [D], F32, kind="ExternalInput").ap()
    out = nc.dram_tensor("out", [N, D], F32, kind="ExternalOutput").ap()
    with tile.TileContext(nc) as tc:
        _emit(tc, out, xT, wq, wk, wv, bq, bk, bv)
    nc.compile()
    return nc


def kernel(**inputs):
    global LAST_RESULT
    x = np.asarray(inputs["x"], np.float32)
    bf = ml_dtypes.bfloat16
    wq = np.asarray(inputs["Wq"], np.float32).astype(bf)
    wk = np.asarray(inputs["Wk"], np.float32).astype(bf)
    wv = np.asarray(inputs["Wv"], np.float32).astype(bf)
    bq = np.ascontiguousarray(np.asarray(inputs["bq"], np.float32))
    bk = np.ascontiguousarray(np.asarray(inputs["bk"], np.float32))
    bv = np.ascontiguousarray(np.asarray(inputs["bv"], np.float32))

    in_maps = []
    for b in range(B):
        in_maps.append(
            {
                "xT": np.ascontiguousarray(x[b].T).astype(bf),
                "wq": wq,
                "wk": wk,
                "wv": wv,
                "bq": bq,
                "bk": bk,
                "bv": bv,
            }
        )

    nc = _build()
    res = bass_utils.run_bass_kernel_spmd(nc, in_maps, core_ids=list(range(B)))
    LAST_RESULT = res
    return np.stack([res.results[c]["out"] for c in range(B)], axis=0)


if __name__ == "__main__":
    rng = np.random.default_rng(0)
    demo = {
        "x": rng.standard_normal((B, N, D), dtype=np.float32),
        "Wq": rng.uniform(-0.036, 0.036, (D, D)).astype(np.float32),
        "bq": rng.uniform(-0.036, 0.036, D).astype(np.float32),
        "Wk": rng.uniform(-0.036, 0.036, (D, D)).astype(np.float32),
        "bk": rng.uniform(-0.036, 0.036, D).astype(np.float32),
        "Wv": rng.uniform(-0.036, 0.036, (D, D)).astype(np.float32),
        "bv": rng.uniform(-0.036, 0.036, D).astype(np.float32),
    }
    out = kernel(**demo)
    print("out", out.shape, out.dtype, float(np.abs(out).max()))
